# revision 19
# baseline (speedup 1.0000x reference)
"""Trainium2 Bass kernel for an 8-head MultiHeadAttention (b=8, s=1024, d=512).

Sharding: pure data-parallel over batch -- each of the 8 NeuronCores runs the
full attention for one batch element. No collectives.

Per-core algorithm (matmul operands bf16, accumulate fp32):
  x^T, w^T built via PE transposes.
  Q^T[hd,s] = wq^T.T @ x^T   (scale 1/8 and bias folded into PSUM->SBUF copy)
  K^T[hd,s] = wk^T.T @ x^T
  V[s,hd]   = x^T.T @ wv^T   (head-interleaved, ones column per head)
  S^T[k,q]  = K_h^T.T @ Q_h^T  -- HEAD-PAIR CONCURRENT via PE 64-row tiling:
              even head on partitions 0:64 (tile 0,0), odd head on 64:128
              (tile 64,0); interleaved emission makes the pairs run
              concurrently on the systolic array.
  P^T       = exp(S^T) * (1-mask)^T  (exp on ACT engine; mask-mul split
              across DVE and GPSIMD)
  O^T_h[65,q] = V_aug.T @ P^T  -- also pair-concurrent: contraction split
              64/64 so the head pair occupies both PE row groups; row 64 =
              softmax denominator via the ones column.
  O^T_h[0:64] *= 1/denom  (reciprocal_approx_fast on DVE, partition
              broadcast on GPSIMD, multiply on DVE)
  out[q,d]  = O^T.T @ wo^T + bo

Mask DMA is split into 8 column strips so (1-mask)^T strips are built
incrementally, letting attention start ~15us into the kernel instead of
waiting for the full setup phase.
"""

import numpy as np

P = 128
S = 1024  # sequence length
D = 512  # d_model
H = 8  # heads
DK = 64  # head dim
CH = D // P  # 4 hd/dmodel chunks
ST = S // P  # 8 seq tiles
NCORES = 8

# mask-mul strips handled by gpsimd (per head, by kc index)
GP_MUL_KC = (2, 5)

# hardware-bisection flags
USE_GP_BCAST = False  # partition_broadcast on gpsimd vs PE f32r outer-product
USE_FAST_RECIP = False  # reciprocal_approx_fast vs vector.reciprocal
PV_PAIR = False  # pair-concurrent half-contraction PV vs sequential

_CACHE = {}


def _build():
    import concourse.bacc as bacc
    import concourse.mybir as mybir
    import concourse.tile as tile
    from concourse.masks import make_identity

    f32 = mybir.dt.float32
    mmdt = mybir.dt.bfloat16
    AF = mybir.ActivationFunctionType
    OP = mybir.AluOpType

    nc = bacc.Bacc(None, target_bir_lowering=False, debug=False)

    x_t = nc.dram_tensor("x", [S, D], f32, kind="ExternalInput")
    mask_t = nc.dram_tensor("mask", [S, S], f32, kind="ExternalInput")
    wq_t = nc.dram_tensor("wq", [D, D], f32, kind="ExternalInput")
    wk_t = nc.dram_tensor("wk", [D, D], f32, kind="ExternalInput")
    wv_t = nc.dram_tensor("wv", [D, D], f32, kind="ExternalInput")
    wo_t = nc.dram_tensor("wo", [D, D], f32, kind="ExternalInput")
    bq_t = nc.dram_tensor("bq", [D], f32, kind="ExternalInput")
    bk_t = nc.dram_tensor("bk", [D], f32, kind="ExternalInput")
    bv_t = nc.dram_tensor("bv", [D], f32, kind="ExternalInput")
    bo_t = nc.dram_tensor("bo", [D], f32, kind="ExternalInput")
    out_t = nc.dram_tensor("out", [S, D], f32, kind="ExternalOutput")

    with tile.TileContext(nc) as tc:
        with (
            tc.tile_pool(name="persist", bufs=1) as pp,
            tc.tile_pool(name="stage", bufs=1) as stage,
            tc.tile_pool(name="ptp", bufs=4) as ptp,
            tc.tile_pool(name="nrm", bufs=2) as nrm,
            tc.tile_pool(name="fin", bufs=3) as fpool,
            tc.tile_pool(name="psc", bufs=2, space="PSUM") as psc,
            tc.tile_pool(name="ppv", bufs=2, space="PSUM") as ppv,
            tc.tile_pool(name="ppr", bufs=1, space="PSUM") as ppr,
        ):
            # ---- constants ----
            ident = pp.tile([P, P], f32, name="id", tag="id")
            make_identity(nc, ident[:])
            ones_f32 = pp.tile([P, P], f32, name="ones_f32", tag="ones_f32")
            nc.vector.memset(ones_f32[:], 1.0)
            ones_sb = pp.tile([1, P], mmdt, name="ones", tag="ones")
            nc.vector.tensor_copy(ones_sb[:], ones_f32[0:1, :])

            bq_sb = pp.tile([P, CH], f32, name="bq", tag="bq")
            bk_sb = pp.tile([P, CH], f32, name="bk", tag="bk")
            nc.sync.dma_start(out=bq_sb[:], in_=bq_t[:].rearrange("(c p) -> p c", p=P))
            nc.sync.dma_start(out=bk_sb[:], in_=bk_t[:].rearrange("(c p) -> p c", p=P))
            qbias_sb = pp.tile([P, CH], f32, name="qbias", tag="qbias")
            nc.vector.tensor_scalar_mul(qbias_sb[:], bq_sb[:], 0.125)

            bv_bc = pp.tile([P, D], f32, name="bvbc", tag="bvbc")
            bo_bc = pp.tile([P, D], f32, name="bobc", tag="bobc")
            nc.gpsimd.dma_start(out=bv_bc[:], in_=bv_t[None, :].to_broadcast([P, D]))
            nc.gpsimd.dma_start(out=bo_bc[:], in_=bo_t[None, :].to_broadcast([P, D]))
            # (gpsimd library loads for partition_broadcast are inserted
            # automatically by Bacc.insert_library_loads)

            # ---- input DMAs (SP queue, in priority order) ----
            xc = []
            for c in range(CH):
                t = stage.tile([P, ST, P], f32, name="xc", tag=f"xc{c}")
                nc.sync.dma_start(
                    out=t[:],
                    in_=x_t[:, c * P : (c + 1) * P].rearrange("(i p) d -> p i d", p=P),
                )
                xc.append(t)
            # weight row-chunk staging: one shared 2-deep ring; the WAR
            # dependencies against the PE transposes sequence the loads.
            # Order on the SP queue: wq, wk, then mask strips, then wv, wo --
            # so nothing the early attention needs sits behind a DMA whose
            # issue WAR-waits on late PE work.
            wc = {}

            def dma_w(name, t):
                wc[name] = []
                for c in range(CH):
                    w = stage.tile([P, CH, P], f32, name="wc", tag="wc", bufs=2)
                    nc.sync.dma_start(
                        out=w[:],
                        in_=t[:, c * P : (c + 1) * P].rearrange(
                            "(r p) d -> p r d", p=P
                        ),
                    )
                    wc[name].append(w)

            dma_w("wq", wq_t)
            dma_w("wk", wk_t)
            # mask column strips (k-blocks)
            msk = []
            for kc in range(ST):
                m = stage.tile([P, ST, P], f32, name="msk", tag="msk", bufs=4)
                nc.sync.dma_start(
                    out=m[:],
                    in_=mask_t[:, kc * P : (kc + 1) * P].rearrange(
                        "(i p) k -> p i k", p=P
                    ),
                )
                msk.append(m)
            dma_w("wv", wv_t)
            dma_w("wo", wo_t)

            # ---- x^T (PE transposes; ACT copies, ramp phase) ----
            xT = stage.tile([P, CH, S], mmdt, name="xT", tag="xT")
            for c in range(CH):
                ps = ppr.tile([P, S], f32, name="pr", tag="pr")
                for i in range(ST):
                    nc.tensor.transpose(
                        ps[:, i * P : (i + 1) * P], xc[c][:, i, :], ident[:]
                    )
                nc.scalar.copy(xT[:, c, :], ps[:])

            # ---- w^T for wq, wk (wv, wo later) ----
            wT = {}

            def build_wT(name, pool):
                wT[name] = pool.tile([P, CH, D], mmdt, name="T", tag="T" + name)
                for c in range(CH):
                    ps = ppr.tile([P, S], f32, name="pr", tag="pr")
                    for rr in range(CH):
                        nc.tensor.transpose(
                            ps[:, rr * P : (rr + 1) * P], wc[name][c][:, rr, :], ident[:]
                        )
                    nc.scalar.copy(wT[name][:, c, :], ps[:, 0:D])

            build_wT("wq", stage)
            build_wT("wk", stage)

            # ---- projections Q^T, K^T chunk-by-chunk ----
            qT = pp.tile([P, CH, S], mmdt, name="qT", tag="qT")
            kT = pp.tile([P, CH, S], mmdt, name="kT", tag="kT")

            def proj_qk(c):
                # chunk c covers heads 2c (partitions 0:64) and 2c+1 (64:128)
                for dst, wname, bias, scale in (
                    (qT, "wq", qbias_sb, 0.125),
                    (kT, "wk", bk_sb, 1.0),
                ):
                    ps = ppr.tile([P, S], f32, name="pr", tag="pr")
                    for j in range(2):
                        for rr in range(CH):
                            nc.tensor.matmul(
                                ps[:, j * 512 : (j + 1) * 512],
                                wT[wname][:, rr, c * P : (c + 1) * P],
                                xT[:, rr, j * 512 : (j + 1) * 512],
                                start=(rr == 0),
                                stop=(rr == CH - 1),
                            )
                    if c < 2:
                        # ramp phase: ACT is free
                        nc.scalar.activation(
                            dst[:, c, :], ps[:], AF.Identity,
                            bias=bias[:, c : c + 1], scale=scale,
                        )
                    else:
                        # attention phase: keep ACT for exp
                        nc.vector.tensor_scalar(
                            dst[:, c, :], ps[:], scale, bias[:, c : c + 1],
                            op0=OP.mult, op1=OP.add,
                        )

            proj_qk(0)
            proj_qk(1)

            # ---- (1-mask)^T strip builder ----
            omT = pp.tile([P, ST, S], mmdt, name="omT", tag="omT")

            def build_om(kc):
                ps = ppr.tile([P, S], f32, name="pr", tag="pr")
                for qi in range(ST):
                    nc.tensor.transpose(
                        ps[:, qi * P : (qi + 1) * P], msk[kc][:, qi, :], ident[:]
                    )
                nc.vector.tensor_scalar(
                    omT[:, kc, :], ps[:], -1.0, 1.0, op0=OP.mult, op1=OP.add
                )

            # ---- attention: scores+exp+mask for a head pair ----
            pts = {}

            def emit_scores(p, with_om=False):
                c = p
                hA, hB = 2 * p, 2 * p + 1
                ptA = ptp.tile([P, ST, S], mmdt, name="pt", tag="pt")
                ptB = ptp.tile([P, ST, S], mmdt, name="pt", tag="pt")
                pts[hA], pts[hB] = ptA, ptB
                qA = qT[0:64, c, :]
                qB = qT[64:128, c, :]
                for kc in range(ST):
                    if with_om:
                        build_om(kc)
                    kA = kT[0:64, c, kc * P : (kc + 1) * P]
                    kB = kT[64:128, c, kc * P : (kc + 1) * P]
                    psA = psc.tile([P, S], f32, name="ps", tag="ps")
                    psB = psc.tile([P, S], f32, name="ps", tag="ps")
                    for j in range(2):
                        nc.tensor.matmul(
                            psA[:, j * 512 : (j + 1) * 512],
                            kA, qA[:, j * 512 : (j + 1) * 512],
                            start=True, stop=True,
                        )
                        nc.tensor.matmul(
                            psB[:, j * 512 : (j + 1) * 512],
                            kB, qB[:, j * 512 : (j + 1) * 512],
                            start=True, stop=True,
                        )
                    nc.scalar.activation(ptA[:, kc, :], psA[:], AF.Exp)
                    nc.scalar.activation(ptB[:, kc, :], psB[:], AF.Exp)
                    eng = nc.gpsimd if kc in GP_MUL_KC else nc.vector
                    eng.tensor_mul(ptA[:, kc, :], ptA[:, kc, :], omT[:, kc, :])
                    eng.tensor_mul(ptB[:, kc, :], ptB[:, kc, :], omT[:, kc, :])

            # ---- attention: PV + normalize for a head pair ----
            def emit_pv(p):
                hA, hB = 2 * p, 2 * p + 1
                c = p
                ptA, ptB = pts.pop(hA), pts.pop(hB)
                vA = v_sb[:].rearrange("p i (h e) -> p i h e", e=65)[:, :, hA, :]
                vB = v_sb[:].rearrange("p i (h e) -> p i h e", e=65)[:, :, hB, :]
                for j in range(2):
                    jsl = slice(j * 512, (j + 1) * 512)
                    pvA = ppv.tile([P, 512], f32, name="pv", tag="pv")
                    pvB = ppv.tile([P, 512], f32, name="pv", tag="pv")
                    if PV_PAIR:
                        # half-contraction split: the pair occupies both PE
                        # row groups -> concurrent execution
                        for kc in range(ST):
                            st = kc == 0
                            sp = kc == ST - 1
                            nc.tensor.matmul(
                                pvA[0:65, :], vA[0:64, kc, :], ptA[0:64, kc, jsl],
                                start=st, stop=False, skip_group_check=True,
                            )
                            nc.tensor.matmul(
                                pvB[0:65, :], vB[64:128, kc, :], ptB[64:128, kc, jsl],
                                start=st, stop=False, skip_group_check=True,
                            )
                            nc.tensor.matmul(
                                pvA[0:65, :], vA[64:128, kc, :], ptA[64:128, kc, jsl],
                                start=False, stop=sp, skip_group_check=True,
                            )
                            nc.tensor.matmul(
                                pvB[0:65, :], vB[0:64, kc, :], ptB[0:64, kc, jsl],
                                start=False, stop=sp, skip_group_check=True,
                            )
                    else:
                        for pv, v, pt in ((pvA, vA, ptA), (pvB, vB, ptB)):
                            for kc in range(ST):
                                nc.tensor.matmul(
                                    pv[0:65, :], v[:, kc, :], pt[:, kc, jsl],
                                    start=(kc == 0), stop=(kc == ST - 1),
                                )
                    # reciprocal of the denominator row, to bf16 for the PE
                    # outer-product broadcast; col groups 0/64 of the bp
                    # psum are written by concurrent matmuls
                    rbs = []
                    for pv in (pvA, pvB):
                        rb = nrm.tile([1, 512], mmdt, name="rb", tag="rb")
                        if USE_FAST_RECIP:
                            rc = nrm.tile([1, 512], f32, name="rc", tag="rc")
                            nc.vector.reciprocal_approx_fast(rc[:], pv[64:65, :])
                            with nc.allow_low_precision(reason="bf16 recip for bf16 mm"):
                                nc.gpsimd.tensor_copy(rb[:], rc[:])
                        else:
                            with nc.allow_low_precision(reason="bf16 recip for bf16 mm"):
                                nc.vector.reciprocal(rb[:], pv[64:65, :])
                        rbs.append(rb)
                    bp = ppr.tile([P, S], f32, name="pr", tag="pr")
                    for bi, rb in enumerate(rbs):
                        nc.tensor.matmul(
                            bp[64 * bi : 64 * bi + 64, 0:512],
                            ones_sb[:, 0:64], rb[:],
                            start=True, stop=True,
                        )
                    for h, pv, bi in ((hA, pvA, 0), (hB, pvB, 1)):
                        off = 64 * (h % 2)
                        bcs = nrm.tile([64, 512], mmdt, name="bcs", tag="bcs")
                        nc.vector.tensor_copy(bcs[:], bp[64 * bi : 64 * bi + 64, 0:512])
                        nc.vector.tensor_mul(
                            oT[off : off + 64, c, jsl], pv[0:64, :], bcs[:]
                        )

            # ---- V projection (head-interleaved + ones col) ----
            v_sb = pp.tile([P, ST, H * 65], mmdt, name="v", tag="v")
            oT = pp.tile([P, CH, S], mmdt, name="oT", tag="oT")

            def proj_v():
                nc.vector.tensor_copy(
                    v_sb[:].rearrange("p i (h e) -> p i h e", e=65)[:, :, :, 64],
                    ones_f32[:, 0 : ST * H].rearrange("p (i h) -> p i h", h=H),
                )
                for i in range(ST):
                    ps = ppr.tile([P, S], f32, name="pr", tag="pr")
                    for rr in range(CH):
                        nc.tensor.matmul(
                            ps[:, 0:512],
                            xT[:, rr, i * P : (i + 1) * P],
                            wT["wv"][:, rr, :],
                            start=(rr == 0),
                            stop=(rr == CH - 1),
                        )
                    nc.vector.tensor_add(
                        v_sb[:, i, :].rearrange("p (h e) -> p h e", e=65)[:, :, 0:64],
                        ps[:, 0:512].rearrange("p (h e) -> p h e", e=64),
                        bv_bc[:].rearrange("p (h e) -> p h e", e=64),
                    )

            # ---- emission schedule (PE queue order) ----
            emit_scores(0, with_om=True)
            build_wT("wv", stage)
            proj_v()
            emit_scores(1)
            proj_qk(2)
            emit_pv(0)
            emit_scores(2)
            proj_qk(3)
            emit_pv(1)
            emit_scores(3)
            build_wT("wo", pp)
            emit_pv(2)
            emit_pv(3)

            # ---- output projection ----
            for qt in range(ST):
                ps = ppr.tile([P, S], f32, name="pr", tag="pr")
                for cc in range(CH):
                    nc.tensor.matmul(
                        ps[:, 0:512],
                        oT[:, cc, qt * P : (qt + 1) * P],
                        wT["wo"][:, cc, :],
                        start=(cc == 0),
                        stop=(cc == CH - 1),
                    )
                ft = fpool.tile([P, 512], f32, name="fin", tag="fin")
                nc.vector.tensor_add(ft[:], ps[:, 0:512], bo_bc[:])
                nc.sync.dma_start(out=out_t[qt * P : (qt + 1) * P, :], in_=ft[:])

    nc.compile()
    return nc


def _get_nc():
    if "nc" not in _CACHE:
        _CACHE["nc"] = _build()
    return _CACHE["nc"]


def run(inputs, trace=False, **kw):
    from concourse.bass_utils import run_bass_kernel_spmd

    nc = _get_nc()
    f = np.float32
    in_maps = [
        {
            "x": np.ascontiguousarray(inputs["inputs"][i], dtype=f),
            "mask": np.ascontiguousarray(inputs["mask"][i], dtype=f),
            "wq": np.ascontiguousarray(inputs["wq"], dtype=f),
            "wk": np.ascontiguousarray(inputs["wk"], dtype=f),
            "wv": np.ascontiguousarray(inputs["wv"], dtype=f),
            "wo": np.ascontiguousarray(inputs["wo"], dtype=f),
            "bq": np.ascontiguousarray(inputs["bq"], dtype=f),
            "bk": np.ascontiguousarray(inputs["bk"], dtype=f),
            "bv": np.ascontiguousarray(inputs["bv"], dtype=f),
            "bo": np.ascontiguousarray(inputs["bo"], dtype=f),
        }
        for i in range(NCORES)
    ]
    res = run_bass_kernel_spmd(nc, in_maps, list(range(NCORES)), trace=trace, **kw)
    out = np.stack(
        [np.asarray(res.results[i]["out"], dtype=np.float32) for i in range(NCORES)],
        axis=0,
    )
    return out, res


def kernel(**inputs):
    out, _ = run(inputs)
    return out


# revision 30
# speedup vs baseline: 1.2451x; 1.2451x over previous
"""Trainium2 Bass kernel for an 8-head MultiHeadAttention (b=8, s=1024, d=512).

Sharding: pure data-parallel over batch -- each of the 8 NeuronCores runs the
full attention for one batch element. No collectives.

Per-core algorithm (matmul operands bf16, accumulate fp32):
  x^T, w^T built via PE transposes.
  Q^T[hd,s] = wq^T.T @ x^T   (scale 1/8 and bias folded into PSUM->SBUF copy)
  K^T[hd,s] = wk^T.T @ x^T
  V[s,hd]   = x^T.T @ wv^T   (head-interleaved, ones column per head)
  S^T[k,q]  = K_h^T.T @ Q_h^T  -- HEAD-PAIR CONCURRENT via PE 64-row tiling:
              even head on partitions 0:64 (tile 0,0), odd head on 64:128
              (tile 64,0); interleaved emission makes the pairs run
              concurrently on the systolic array.
  P^T       = exp(S^T) * (1-mask)^T  (exp on ACT engine; mask-mul split
              across DVE and GPSIMD)
  O^T_h[65,q] = V_aug.T @ P^T  -- also pair-concurrent: contraction split
              64/64 so the head pair occupies both PE row groups; row 64 =
              softmax denominator via the ones column.
  O^T_h[0:64] *= 1/denom  (reciprocal_approx_fast on DVE, partition
              broadcast on GPSIMD, multiply on DVE)
  out[q,d]  = O^T.T @ wo^T + bo

Mask DMA is split into 8 column strips so (1-mask)^T strips are built
incrementally, letting attention start ~15us into the kernel instead of
waiting for the full setup phase.
"""

import numpy as np

P = 128
S = 1024  # sequence length
D = 512  # d_model
H = 8  # heads
DK = 64  # head dim
CH = D // P  # 4 hd/dmodel chunks
ST = S // P  # 8 seq tiles
NCORES = 8

# mask-mul strips handled by gpsimd (per head, by kc index)
GP_MUL_KC = (2, 5, 7)

# hardware-bisection flags
USE_GP_BCAST = False  # partition_broadcast on gpsimd vs PE f32r outer-product
USE_FAST_RECIP = True  # reciprocal_approx_fast vs vector.reciprocal
PV_PAIR = False  # pair-concurrent half-contraction PV vs sequential

_CACHE = {}


def _build():
    import concourse.bacc as bacc
    import concourse.mybir as mybir
    import concourse.tile as tile
    from concourse.masks import make_identity

    f32 = mybir.dt.float32
    mmdt = mybir.dt.bfloat16
    AF = mybir.ActivationFunctionType
    OP = mybir.AluOpType

    nc = bacc.Bacc(None, target_bir_lowering=False, debug=False)

    x_t = nc.dram_tensor("x", [S, D], f32, kind="ExternalInput")
    mask_t = nc.dram_tensor("mask", [S, S], f32, kind="ExternalInput")
    wq_t = nc.dram_tensor("wq", [D, D], f32, kind="ExternalInput")
    wk_t = nc.dram_tensor("wk", [D, D], f32, kind="ExternalInput")
    wv_t = nc.dram_tensor("wv", [D, D], f32, kind="ExternalInput")
    wo_t = nc.dram_tensor("wo", [D, D], f32, kind="ExternalInput")
    bq_t = nc.dram_tensor("bq", [D], f32, kind="ExternalInput")
    bk_t = nc.dram_tensor("bk", [D], f32, kind="ExternalInput")
    bv_t = nc.dram_tensor("bv", [D], f32, kind="ExternalInput")
    bo_t = nc.dram_tensor("bo", [D], f32, kind="ExternalInput")
    out_t = nc.dram_tensor("out", [S, D], f32, kind="ExternalOutput")

    with tile.TileContext(nc) as tc:
        with (
            tc.tile_pool(name="persist", bufs=1) as pp,
            tc.tile_pool(name="stage", bufs=1) as stage,
            tc.tile_pool(name="ptp", bufs=4) as ptp,
            tc.tile_pool(name="nrm", bufs=2) as nrm,
            tc.tile_pool(name="fin", bufs=3) as fpool,
            tc.tile_pool(name="psc", bufs=2, space="PSUM") as psc,
            tc.tile_pool(name="ppv", bufs=2, space="PSUM") as ppv,
            tc.tile_pool(name="ppr", bufs=1, space="PSUM") as ppr,
        ):
            # ---- constants ----
            ident = pp.tile([P, P], f32, name="id", tag="id")
            make_identity(nc, ident[:])
            ones_f32 = pp.tile([P, P], f32, name="ones_f32", tag="ones_f32")
            nc.vector.memset(ones_f32[:], 1.0)
            ones_sb = pp.tile([1, P], mmdt, name="ones", tag="ones")
            nc.vector.tensor_copy(ones_sb[:], ones_f32[0:1, :])
            # indicator for the recip broadcast: denominator slot i lives on
            # partition 32*i; for j-slice, out rows 0:64 take slot 2j and
            # rows 64:128 take slot 2j+1
            e4 = pp.tile([P, 2 * P], mmdt, name="e4", tag="e4")
            nc.vector.memset(e4[:], 0.0)
            for j in range(2):
                nc.vector.memset(
                    e4[32 * 2 * j : 32 * 2 * j + 1, j * P : j * P + 64], 1.0
                )
                nc.vector.memset(
                    e4[32 * (2 * j + 1) : 32 * (2 * j + 1) + 1,
                       j * P + 64 : (j + 1) * P], 1.0
                )

            bq_sb = pp.tile([P, CH], f32, name="bq", tag="bq")
            bk_sb = pp.tile([P, CH], f32, name="bk", tag="bk")
            nc.sync.dma_start(out=bq_sb[:], in_=bq_t[:].rearrange("(c p) -> p c", p=P))
            nc.sync.dma_start(out=bk_sb[:], in_=bk_t[:].rearrange("(c p) -> p c", p=P))
            qbias_sb = pp.tile([P, CH], f32, name="qbias", tag="qbias")
            nc.vector.tensor_scalar_mul(qbias_sb[:], bq_sb[:], 0.125)

            bv_bc = pp.tile([P, D], f32, name="bvbc", tag="bvbc")
            bo_bc = pp.tile([P, D], f32, name="bobc", tag="bobc")
            nc.gpsimd.dma_start(out=bv_bc[:], in_=bv_t[None, :].to_broadcast([P, D]))
            nc.gpsimd.dma_start(out=bo_bc[:], in_=bo_t[None, :].to_broadcast([P, D]))
            # (gpsimd library loads for partition_broadcast are inserted
            # automatically by Bacc.insert_library_loads)

            # ---- input DMAs (SP queue, in priority order) ----
            xc = []
            for c in range(CH):
                t = stage.tile([P, ST, P], f32, name="xc", tag=f"xc{c}")
                nc.sync.dma_start(
                    out=t[:],
                    in_=x_t[:, c * P : (c + 1) * P].rearrange("(i p) d -> p i d", p=P),
                )
                xc.append(t)
            # weight row-chunk staging: one shared 2-deep ring; the WAR
            # dependencies against the PE transposes sequence the loads.
            # Order on the SP queue: wq, wk, then mask strips, then wv, wo --
            # so nothing the early attention needs sits behind a DMA whose
            # issue WAR-waits on late PE work.
            wc = {}

            def dma_w(name, t):
                wc[name] = []
                for c in range(CH):
                    w = stage.tile([P, CH, P], f32, name="wc", tag="wc", bufs=2)
                    nc.sync.dma_start(
                        out=w[:],
                        in_=t[:, c * P : (c + 1) * P].rearrange(
                            "(r p) d -> p r d", p=P
                        ),
                    )
                    wc[name].append(w)

            dma_w("wq", wq_t)
            dma_w("wk", wk_t)
            # mask column strips (k-blocks)
            msk = []
            for kc in range(ST):
                m = stage.tile([P, ST, P], f32, name="msk", tag="msk", bufs=4)
                nc.sync.dma_start(
                    out=m[:],
                    in_=mask_t[:, kc * P : (kc + 1) * P].rearrange(
                        "(i p) k -> p i k", p=P
                    ),
                )
                msk.append(m)
            dma_w("wv", wv_t)
            dma_w("wo", wo_t)

            # ---- x^T (PE transposes; ACT copies, ramp phase) ----
            xT = stage.tile([P, CH, S], mmdt, name="xT", tag="xT")
            for c in range(CH):
                ps = ppr.tile([P, S], f32, name="pr", tag="pr")
                for i in range(ST):
                    nc.tensor.transpose(
                        ps[:, i * P : (i + 1) * P], xc[c][:, i, :], ident[:]
                    )
                nc.scalar.copy(xT[:, c, :], ps[:])

            # ---- w^T for wq, wk (wv, wo later) ----
            wT = {}

            def build_wT(name, pool):
                wT[name] = pool.tile([P, CH, D], mmdt, name="T", tag="T" + name)
                for c in range(CH):
                    ps = ppr.tile([P, S], f32, name="pr", tag="pr")
                    for rr in range(CH):
                        nc.tensor.transpose(
                            ps[:, rr * P : (rr + 1) * P], wc[name][c][:, rr, :], ident[:]
                        )
                    nc.scalar.copy(wT[name][:, c, :], ps[:, 0:D])

            build_wT("wq", stage)
            build_wT("wk", stage)

            # ---- projections Q^T, K^T chunk-by-chunk ----
            qT = pp.tile([P, CH, S], mmdt, name="qT", tag="qT")
            kT = pp.tile([P, CH, S], mmdt, name="kT", tag="kT")

            def proj_qk(c):
                # chunk c covers heads 2c (partitions 0:64) and 2c+1 (64:128)
                for dst, wname, bias, scale in (
                    (qT, "wq", qbias_sb, 0.125),
                    (kT, "wk", bk_sb, 1.0),
                ):
                    ps = ppr.tile([P, S], f32, name="pr", tag="pr")
                    for j in range(2):
                        for rr in range(CH):
                            nc.tensor.matmul(
                                ps[:, j * 512 : (j + 1) * 512],
                                wT[wname][:, rr, c * P : (c + 1) * P],
                                xT[:, rr, j * 512 : (j + 1) * 512],
                                start=(rr == 0),
                                stop=(rr == CH - 1),
                            )
                    nc.scalar.activation(
                        dst[:, c, :], ps[:], AF.Identity,
                        bias=bias[:, c : c + 1], scale=scale,
                    )

            proj_qk(0)
            proj_qk(1)

            # ---- (1-mask)^T strip builder ----
            omT = pp.tile([P, ST, S], mmdt, name="omT", tag="omT")

            def build_om(kc):
                ps = ppr.tile([P, S], f32, name="pr", tag="pr")
                for qi in range(ST):
                    nc.tensor.transpose(
                        ps[:, qi * P : (qi + 1) * P], msk[kc][:, qi, :], ident[:]
                    )
                nc.vector.tensor_scalar(
                    omT[:, kc, :], ps[:], -1.0, 1.0, op0=OP.mult, op1=OP.add
                )

            # ---- attention: scores+exp+mask for a head pair ----
            pts = {}

            def emit_scores(p, with_om=False):
                c = p
                hA, hB = 2 * p, 2 * p + 1
                ptA = ptp.tile([P, ST, S], mmdt, name="pt", tag="pt")
                ptB = ptp.tile([P, ST, S], mmdt, name="pt", tag="pt")
                pts[hA], pts[hB] = ptA, ptB
                qA = qT[0:64, c, :]
                qB = qT[64:128, c, :]
                for kc in range(ST):
                    if with_om:
                        build_om(kc)
                    kA = kT[0:64, c, kc * P : (kc + 1) * P]
                    kB = kT[64:128, c, kc * P : (kc + 1) * P]
                    psA = psc.tile([P, S], f32, name="ps", tag="ps")
                    psB = psc.tile([P, S], f32, name="ps", tag="ps")
                    for j in range(2):
                        nc.tensor.matmul(
                            psA[:, j * 512 : (j + 1) * 512],
                            kA, qA[:, j * 512 : (j + 1) * 512],
                            start=True, stop=True,
                        )
                        nc.tensor.matmul(
                            psB[:, j * 512 : (j + 1) * 512],
                            kB, qB[:, j * 512 : (j + 1) * 512],
                            start=True, stop=True,
                        )
                    nc.scalar.activation(ptA[:, kc, :], psA[:], AF.Exp)
                    nc.scalar.activation(ptB[:, kc, :], psB[:], AF.Exp)
                    eng = nc.gpsimd if kc in GP_MUL_KC else nc.vector
                    eng.tensor_mul(ptA[:, kc, :], ptA[:, kc, :], omT[:, kc, :])
                    eng.tensor_mul(ptB[:, kc, :], ptB[:, kc, :], omT[:, kc, :])

            # ---- attention: PV + normalize for a head pair ----
            def emit_pv(p):
                hA, hB = 2 * p, 2 * p + 1
                c = p
                ptA, ptB = pts.pop(hA), pts.pop(hB)
                vA = v_sb[:].rearrange("p i (h e) -> p i h e", e=65)[:, :, hA, :]
                vB = v_sb[:].rearrange("p i (h e) -> p i h e", e=65)[:, :, hB, :]
                dn = nrm.tile([P, 512], f32, name="dn", tag="dn")
                nc.vector.memset(dn[:], 1.0)
                for j in range(2):
                    jsl = slice(j * 512, (j + 1) * 512)
                    pvA = ppv.tile([P, 512], f32, name="pv", tag="pv")
                    pvB = ppv.tile([P, 512], f32, name="pv", tag="pv")
                    if PV_PAIR:
                        # half-contraction split: the pair occupies both PE
                        # row groups -> concurrent execution
                        for kc in range(ST):
                            st = kc == 0
                            sp = kc == ST - 1
                            nc.tensor.matmul(
                                pvA[0:65, :], vA[0:64, kc, :], ptA[0:64, kc, jsl],
                                start=st, stop=False, skip_group_check=True,
                            )
                            nc.tensor.matmul(
                                pvB[0:65, :], vB[64:128, kc, :], ptB[64:128, kc, jsl],
                                start=st, stop=False, skip_group_check=True,
                            )
                            nc.tensor.matmul(
                                pvA[0:65, :], vA[64:128, kc, :], ptA[64:128, kc, jsl],
                                start=False, stop=sp, skip_group_check=True,
                            )
                            nc.tensor.matmul(
                                pvB[0:65, :], vB[0:64, kc, :], ptB[0:64, kc, jsl],
                                start=False, stop=sp, skip_group_check=True,
                            )
                    else:
                        for pv, v, pt in ((pvA, vA, ptA), (pvB, vB, ptB)):
                            for kc in range(ST):
                                nc.tensor.matmul(
                                    pv[0:65, :], v[:, kc, :], pt[:, kc, jsl],
                                    start=(kc == 0), stop=(kc == ST - 1),
                                )
                    # drain unnormalized O^T and the denominator row; frees
                    # the pv psum ring immediately
                    for idx, (h, pv) in enumerate(((hA, pvA), (hB, pvB))):
                        off = 64 * (h % 2)
                        slot = 32 * (2 * j + idx)
                        nc.vector.tensor_copy(oT[off : off + 64, c, jsl], pv[0:64, :])
                        nc.vector.tensor_copy(
                            dn[slot : slot + 1, :], pv[64:65, :]
                        )
                # one batched reciprocal for the pair's 4 denominator rows
                # (DVE reciprocal cost is free-dim-bound: [128,512] costs
                # the same as [1,512]; non-slot lanes hold 1.0)
                rc4 = nrm.tile([P, 512], f32, name="rc4", tag="rc4")
                nc.vector.reciprocal(rc4[:], dn[:])
                rb4 = nrm.tile([P, 512], mmdt, name="rb4", tag="rb4")
                with nc.allow_low_precision(reason="bf16 recip feeds bf16 matmul"):
                    nc.vector.tensor_copy(rb4[:], rc4[:])
                for j in range(2):
                    jsl = slice(j * 512, (j + 1) * 512)
                    bp = ppr.tile([P, S], f32, name="pr", tag="pr")
                    nc.tensor.matmul(
                        bp[:, 0:512], e4[:, j * P : (j + 1) * P], rb4[:],
                        start=True, stop=True,
                    )
                    for idx, h in enumerate((hA, hB)):
                        off = 64 * (h % 2)
                        osl = oT[off : off + 64, c, jsl]
                        nc.vector.tensor_mul(
                            osl, osl, bp[64 * idx : 64 * idx + 64, 0:512]
                        )

            # ---- V projection (head-interleaved + ones col) ----
            v_sb = pp.tile([P, ST, H * 65], mmdt, name="v", tag="v")
            oT = pp.tile([P, CH, S], mmdt, name="oT", tag="oT")

            def proj_v():
                nc.vector.tensor_copy(
                    v_sb[:].rearrange("p i (h e) -> p i h e", e=65)[:, :, :, 64],
                    ones_f32[:, 0 : ST * H].rearrange("p (i h) -> p i h", h=H),
                )
                for i in range(ST):
                    ps = ppr.tile([P, S], f32, name="pr", tag="pr")
                    for rr in range(CH):
                        nc.tensor.matmul(
                            ps[:, 0:512],
                            xT[:, rr, i * P : (i + 1) * P],
                            wT["wv"][:, rr, :],
                            start=(rr == 0),
                            stop=(rr == CH - 1),
                        )
                    nc.vector.tensor_add(
                        v_sb[:, i, :].rearrange("p (h e) -> p h e", e=65)[:, :, 0:64],
                        ps[:, 0:512].rearrange("p (h e) -> p h e", e=64),
                        bv_bc[:].rearrange("p (h e) -> p h e", e=64),
                    )

            # ---- emission schedule (PE queue order) ----
            emit_scores(0, with_om=True)
            build_wT("wv", stage)
            proj_v()
            emit_scores(1)
            proj_qk(2)
            emit_pv(0)
            emit_scores(2)
            proj_qk(3)
            build_wT("wo", pp)
            emit_pv(1)
            emit_scores(3)
            emit_pv(2)
            emit_pv(3)

            # ---- output projection (psc pool is free by now; two q-tiles
            # per 2-bank tile, double-buffered so PE/DVE/DMA pipeline) ----
            for qt in range(ST):
                if qt % 2 == 0:
                    ps = psc.tile([P, S], f32, name="ps", tag="ps")
                half = ps[:, (qt % 2) * 512 : (qt % 2) * 512 + 512]
                for cc in range(CH):
                    nc.tensor.matmul(
                        half,
                        oT[:, cc, qt * P : (qt + 1) * P],
                        wT["wo"][:, cc, :],
                        start=(cc == 0),
                        stop=(cc == CH - 1),
                    )
                ft = fpool.tile([P, 512], f32, name="fin", tag="fin")
                nc.vector.tensor_add(ft[:], half, bo_bc[:])
                nc.sync.dma_start(out=out_t[qt * P : (qt + 1) * P, :], in_=ft[:])

    nc.compile()
    return nc


def _get_nc():
    if "nc" not in _CACHE:
        _CACHE["nc"] = _build()
    return _CACHE["nc"]


def run(inputs, trace=False, **kw):
    from concourse.bass_utils import run_bass_kernel_spmd

    nc = _get_nc()
    f = np.float32
    in_maps = [
        {
            "x": np.ascontiguousarray(inputs["inputs"][i], dtype=f),
            "mask": np.ascontiguousarray(inputs["mask"][i], dtype=f),
            "wq": np.ascontiguousarray(inputs["wq"], dtype=f),
            "wk": np.ascontiguousarray(inputs["wk"], dtype=f),
            "wv": np.ascontiguousarray(inputs["wv"], dtype=f),
            "wo": np.ascontiguousarray(inputs["wo"], dtype=f),
            "bq": np.ascontiguousarray(inputs["bq"], dtype=f),
            "bk": np.ascontiguousarray(inputs["bk"], dtype=f),
            "bv": np.ascontiguousarray(inputs["bv"], dtype=f),
            "bo": np.ascontiguousarray(inputs["bo"], dtype=f),
        }
        for i in range(NCORES)
    ]
    res = run_bass_kernel_spmd(nc, in_maps, list(range(NCORES)), trace=trace, **kw)
    out = np.stack(
        [np.asarray(res.results[i]["out"], dtype=np.float32) for i in range(NCORES)],
        axis=0,
    )
    return out, res


def kernel(**inputs):
    out, _ = run(inputs)
    return out


# revision 33
# speedup vs baseline: 1.3592x; 1.0917x over previous
"""Trainium2 Bass kernel for an 8-head MultiHeadAttention (b=8, s=1024, d=512).

Sharding: pure data-parallel over batch -- each of the 8 NeuronCores runs the
full attention for one batch element. No collectives.

Per-core algorithm (matmul operands bf16, accumulate fp32):
  x^T, w^T built via PE transposes (double-buffered through the score psum
  pool during the ramp).
  Q^T[hd,s] = wq^T.T @ x^T   (scale 1/8 + bias folded into the PSUM drain)
  K^T[hd,s] = wk^T.T @ x^T
  V[s,hd]   = x^T.T @ wv^T   (head-interleaved, ones column per head)
  S^T[k,q]  = K_h^T.T @ Q_h^T  -- head-pair concurrent via PE 64-row tiling:
              even head on partitions 0:64 (tile 0,0), odd head on 64:128
              (tile 64,0); interleaved emission runs the pair concurrently
              on the systolic array.
  P^T       = exp(S^T) * (1-mask)^T  (exp on ACT; mask-mul split DVE/GPSIMD)
  O^T_h[65,q] = V_aug.T @ P^T  (row 64 = softmax denominator via ones col)
  normalize: drain O^T unnormalized + denominator rows to SBUF, one batched
              reciprocal per head pair (denominators parked on partitions
              0/32/64/96 -- DVE reciprocal cost is free-dim-bound), indicator
              matmul broadcasts the recips, in-place multiply on oT.
  out[q,d]  = O^T.T @ wo^T + bo

Schedule: software-pipelined at kc granularity -- scores/exp/mask of pair p
are interleaved instruction-by-instruction with the PV matmuls of pair p-1
plus background work (V projection, later QK chunks, wo^T build), keeping
the ACT-engine exp chain (the throughput bound) saturated. Mask arrives as
8 column strips via SWDGE so (1-mask)^T builds incrementally during pair 0.
"""

import numpy as np

P = 128
S = 1024  # sequence length
D = 512  # d_model
H = 8  # heads
DK = 64  # head dim
CH = D // P  # 4 hd/dmodel chunks
ST = S // P  # 8 seq tiles
NCORES = 8

# mask-mul strips handled by gpsimd (per head, by kc index)
GP_MUL_KC = (2, 5, 7)

_CACHE = {}


def _build():
    import concourse.bacc as bacc
    import concourse.mybir as mybir
    import concourse.tile as tile
    from concourse.masks import make_identity

    f32 = mybir.dt.float32
    mmdt = mybir.dt.bfloat16
    AF = mybir.ActivationFunctionType
    OP = mybir.AluOpType

    nc = bacc.Bacc(None, target_bir_lowering=False, debug=False)

    x_t = nc.dram_tensor("x", [S, D], f32, kind="ExternalInput")
    mask_t = nc.dram_tensor("mask", [S, S], f32, kind="ExternalInput")
    wq_t = nc.dram_tensor("wq", [D, D], f32, kind="ExternalInput")
    wk_t = nc.dram_tensor("wk", [D, D], f32, kind="ExternalInput")
    wv_t = nc.dram_tensor("wv", [D, D], f32, kind="ExternalInput")
    wo_t = nc.dram_tensor("wo", [D, D], f32, kind="ExternalInput")
    bq_t = nc.dram_tensor("bq", [D], f32, kind="ExternalInput")
    bk_t = nc.dram_tensor("bk", [D], f32, kind="ExternalInput")
    bv_t = nc.dram_tensor("bv", [D], f32, kind="ExternalInput")
    bo_t = nc.dram_tensor("bo", [D], f32, kind="ExternalInput")
    out_t = nc.dram_tensor("out", [S, D], f32, kind="ExternalOutput")

    with tile.TileContext(nc) as tc:
        with (
            tc.tile_pool(name="persist", bufs=1) as pp,
            tc.tile_pool(name="stage", bufs=1) as stage,
            tc.tile_pool(name="ptp", bufs=4) as ptp,
            tc.tile_pool(name="nrm", bufs=2) as nrm,
            tc.tile_pool(name="fin", bufs=3) as fpool,
            tc.tile_pool(name="psc", bufs=2, space="PSUM") as psc,
            tc.tile_pool(name="ppv", bufs=2, space="PSUM") as ppv,
            tc.tile_pool(name="ppr", bufs=1, space="PSUM") as ppr,
        ):
            # ---- constants ----
            ident = pp.tile([P, P], f32, name="id", tag="id")
            make_identity(nc, ident[:])
            ones_f32 = pp.tile([P, P], f32, name="ones_f32", tag="ones_f32")
            nc.vector.memset(ones_f32[:], 1.0)
            # indicator for the recip broadcast: denominator slot i lives on
            # partition 32*i; for j-slice, out rows 0:64 take slot 2j and
            # rows 64:128 take slot 2j+1
            e4 = pp.tile([P, 2 * P], mmdt, name="e4", tag="e4")
            nc.vector.memset(e4[:], 0.0)
            for j in range(2):
                nc.vector.memset(
                    e4[32 * 2 * j : 32 * 2 * j + 1, j * P : j * P + 64], 1.0
                )
                nc.vector.memset(
                    e4[32 * (2 * j + 1) : 32 * (2 * j + 1) + 1,
                       j * P + 64 : (j + 1) * P], 1.0
                )

            bq_sb = pp.tile([P, CH], f32, name="bq", tag="bq")
            bk_sb = pp.tile([P, CH], f32, name="bk", tag="bk")
            nc.sync.dma_start(out=bq_sb[:], in_=bq_t[:].rearrange("(c p) -> p c", p=P))
            nc.sync.dma_start(out=bk_sb[:], in_=bk_t[:].rearrange("(c p) -> p c", p=P))
            qbias_sb = pp.tile([P, CH], f32, name="qbias", tag="qbias")
            nc.vector.tensor_scalar_mul(qbias_sb[:], bq_sb[:], 0.125)

            bv_bc = pp.tile([P, D], f32, name="bvbc", tag="bvbc")
            bo_bc = pp.tile([P, D], f32, name="bobc", tag="bobc")
            nc.gpsimd.dma_start(out=bv_bc[:], in_=bv_t[None, :].to_broadcast([P, D]))
            nc.gpsimd.dma_start(out=bo_bc[:], in_=bo_t[None, :].to_broadcast([P, D]))

            # ---- input DMAs ----
            # x + weights on the SP HWDGE queue
            xc = []
            for c in range(CH):
                t = stage.tile([P, ST, P], f32, name="xc", tag=f"xc{c}")
                nc.sync.dma_start(
                    out=t[:],
                    in_=x_t[:, c * P : (c + 1) * P].rearrange("(i p) d -> p i d", p=P),
                )
                xc.append(t)
            wc = {}

            def dma_w(name, t):
                wc[name] = []
                for c in range(CH):
                    w = stage.tile([P, CH, P], f32, name="wc", tag="wc", bufs=6)
                    nc.sync.dma_start(
                        out=w[:],
                        in_=t[:, c * P : (c + 1) * P].rearrange(
                            "(r p) d -> p r d", p=P
                        ),
                    )
                    wc[name].append(w)

            dma_w("wq", wq_t)
            dma_w("wk", wk_t)
            dma_w("wv", wv_t)
            dma_w("wo", wo_t)
            # mask column strips via SWDGE (gpsimd) -- own descriptor path,
            # doesn't contend with the shared DGE block
            msk = []
            for kc in range(ST):
                m = stage.tile([P, ST, P], f32, name="msk", tag="msk", bufs=4)
                nc.gpsimd.dma_start(
                    out=m[:],
                    in_=mask_t[:, kc * P : (kc + 1) * P].rearrange(
                        "(i p) k -> p i k", p=P
                    ),
                )
                msk.append(m)

            # ---- ramp: x^T and wq^T/wk^T via the double-buffered psc ring ----
            xT = stage.tile([P, CH, S], mmdt, name="xT", tag="xT")
            for c in range(CH):
                ps = psc.tile([P, S], f32, name="ps", tag="ps")
                for i in range(ST):
                    nc.tensor.transpose(
                        ps[:, i * P : (i + 1) * P], xc[c][:, i, :], ident[:]
                    )
                nc.scalar.copy(xT[:, c, :], ps[:])

            wT = {}

            def build_wT(name, pool, psum_pool, drain):
                wT[name] = pool.tile([P, CH, D], mmdt, name="T", tag="T" + name)
                for c in range(CH):
                    build_wT_chunk(name, c, psum_pool, drain)

            def build_wT_chunk(name, c, psum_pool, drain):
                ps = psum_pool.tile(
                    [P, S], f32, name="ps" if psum_pool is psc else "pr",
                    tag="ps" if psum_pool is psc else "pr",
                )
                for rr in range(CH):
                    nc.tensor.transpose(
                        ps[:, rr * P : (rr + 1) * P], wc[name][c][:, rr, :], ident[:]
                    )
                drain(wT[name][:, c, :], ps[:, 0:D])

            build_wT("wq", stage, psc, nc.scalar.copy)
            build_wT("wk", stage, psc, nc.scalar.copy)

            # ---- projections Q^T, K^T ----
            qT = pp.tile([P, CH, S], mmdt, name="qT", tag="qT")
            kT = pp.tile([P, CH, S], mmdt, name="kT", tag="kT")

            def proj_qk_dst(c, dst, wname, bias, scale, psum_pool, on_act):
                ps = psum_pool.tile(
                    [P, S], f32, name="ps" if psum_pool is psc else "pr",
                    tag="ps" if psum_pool is psc else "pr",
                )
                for j in range(2):
                    for rr in range(CH):
                        nc.tensor.matmul(
                            ps[:, j * 512 : (j + 1) * 512],
                            wT[wname][:, rr, c * P : (c + 1) * P],
                            xT[:, rr, j * 512 : (j + 1) * 512],
                            start=(rr == 0),
                            stop=(rr == CH - 1),
                        )
                if on_act:
                    nc.scalar.activation(
                        dst[:, c, :], ps[:], AF.Identity,
                        bias=bias[:, c : c + 1], scale=scale,
                    )
                else:
                    nc.vector.tensor_scalar(
                        dst[:, c, :], ps[:], scale, bias[:, c : c + 1],
                        op0=OP.mult, op1=OP.add,
                    )

            for c in (0, 1):
                proj_qk_dst(c, qT, "wq", qbias_sb, 0.125, psc, True)
                proj_qk_dst(c, kT, "wk", bk_sb, 1.0, psc, True)

            # ---- persistent attention state ----
            omT = pp.tile([P, ST, S], mmdt, name="omT", tag="omT")
            v_sb = pp.tile([P, ST, H * 65], mmdt, name="v", tag="v")
            oT = pp.tile([P, CH, S], mmdt, name="oT", tag="oT")
            # homes for the wv/wo transposed weights (chunks built inside
            # the attention loop via build_wT_chunk)
            wT["wv"] = stage.tile([P, CH, D], mmdt, name="T", tag="Twv")
            wT["wo"] = pp.tile([P, CH, D], mmdt, name="T", tag="Two")

            def build_om(kc):
                ps = ppr.tile([P, S], f32, name="pr", tag="pr")
                for qi in range(ST):
                    nc.tensor.transpose(
                        ps[:, qi * P : (qi + 1) * P], msk[kc][:, qi, :], ident[:]
                    )
                nc.vector.tensor_scalar(
                    omT[:, kc, :], ps[:], -1.0, 1.0, op0=OP.mult, op1=OP.add
                )

            def proj_v_unit(i):
                ps = ppr.tile([P, S], f32, name="pr", tag="pr")
                for rr in range(CH):
                    nc.tensor.matmul(
                        ps[:, 0:512],
                        xT[:, rr, i * P : (i + 1) * P],
                        wT["wv"][:, rr, :],
                        start=(rr == 0),
                        stop=(rr == CH - 1),
                    )
                nc.vector.tensor_add(
                    v_sb[:, i, :].rearrange("p (h e) -> p h e", e=65)[:, :, 0:64],
                    ps[:, 0:512].rearrange("p (h e) -> p h e", e=64),
                    bv_bc[:].rearrange("p (h e) -> p h e", e=64),
                )

            # ---- pipelined attention ----
            pts = {}
            pvs = {}

            def scores_unit(p, kc):
                c = p
                ptA, ptB = pts[2 * p], pts[2 * p + 1]
                kA = kT[0:64, c, kc * P : (kc + 1) * P]
                kB = kT[64:128, c, kc * P : (kc + 1) * P]
                psA = psc.tile([P, S], f32, name="ps", tag="ps")
                psB = psc.tile([P, S], f32, name="ps", tag="ps")
                for j in range(2):
                    nc.tensor.matmul(
                        psA[:, j * 512 : (j + 1) * 512],
                        kA, qT[0:64, c, j * 512 : (j + 1) * 512],
                        start=True, stop=True,
                    )
                    nc.tensor.matmul(
                        psB[:, j * 512 : (j + 1) * 512],
                        kB, qT[64:128, c, j * 512 : (j + 1) * 512],
                        start=True, stop=True,
                    )
                nc.scalar.activation(ptA[:, kc, :], psA[:], AF.Exp)
                nc.scalar.activation(ptB[:, kc, :], psB[:], AF.Exp)
                eng = nc.gpsimd if kc in GP_MUL_KC else nc.vector
                eng.tensor_mul(ptA[:, kc, :], ptA[:, kc, :], omT[:, kc, :])
                eng.tensor_mul(ptB[:, kc, :], ptB[:, kc, :], omT[:, kc, :])

            def pv_mm_unit(p, j, kc_pair):
                # two consecutive kc accumulation steps of PV for both heads
                hA, hB = 2 * p, 2 * p + 1
                ptA, ptB = pts[hA], pts[hB]
                vA = v_sb[:].rearrange("p i (h e) -> p i h e", e=65)[:, :, hA, :]
                vB = v_sb[:].rearrange("p i (h e) -> p i h e", e=65)[:, :, hB, :]
                jsl = slice(j * 512, (j + 1) * 512)
                pvA, pvB = pvs[(p, j)]
                for kc in (2 * kc_pair, 2 * kc_pair + 1):
                    st = kc == 0
                    sp = kc == ST - 1
                    nc.tensor.matmul(
                        pvA[0:65, :], vA[:, kc, :], ptA[:, kc, jsl],
                        start=st, stop=sp,
                    )
                    nc.tensor.matmul(
                        pvB[0:65, :], vB[:, kc, :], ptB[:, kc, jsl],
                        start=st, stop=sp,
                    )

            def pv_start(p, j):
                pvA = ppv.tile([P, 512], f32, name="pv", tag="pv")
                pvB = ppv.tile([P, 512], f32, name="pv", tag="pv")
                pvs[(p, j)] = (pvA, pvB)

            def pv_drain(p, j, dn):
                c = p
                hA, hB = 2 * p, 2 * p + 1
                jsl = slice(j * 512, (j + 1) * 512)
                pvA, pvB = pvs.pop((p, j))
                for idx, (h, pv) in enumerate(((hA, pvA), (hB, pvB))):
                    off = 64 * (h % 2)
                    slot = 32 * (2 * j + idx)
                    nc.vector.tensor_copy(oT[off : off + 64, c, jsl], pv[0:64, :])
                    nc.vector.tensor_copy(dn[slot : slot + 1, :], pv[64:65, :])

            def pv_norm(p, dn):
                # one batched reciprocal for the pair's 4 denominator rows
                # (cost is free-dim-bound; non-slot lanes hold 1.0)
                c = p
                hA, hB = 2 * p, 2 * p + 1
                rc4 = nrm.tile([P, 512], f32, name="rc4", tag="rc4")
                nc.vector.reciprocal(rc4[:], dn[:])
                rb4 = nrm.tile([P, 512], mmdt, name="rb4", tag="rb4")
                with nc.allow_low_precision(reason="bf16 recip feeds bf16 matmul"):
                    nc.vector.tensor_copy(rb4[:], rc4[:])
                for j in range(2):
                    jsl = slice(j * 512, (j + 1) * 512)
                    bp = ppr.tile([P, S], f32, name="pr", tag="pr")
                    nc.tensor.matmul(
                        bp[:, 0:512], e4[:, j * P : (j + 1) * P], rb4[:],
                        start=True, stop=True,
                    )
                    for idx, h in enumerate((hA, hB)):
                        off = 64 * (h % 2)
                        osl = oT[off : off + 64, c, jsl]
                        nc.vector.tensor_mul(
                            osl, osl, bp[64 * idx : 64 * idx + 64, 0:512]
                        )

            def dve_drain(dst, src):
                nc.vector.tensor_copy(dst, src)

            # pair p scores run with pair p-1 PV interleaved at kc granularity
            dns = {}
            for p in range(H // 2):
                pts[2 * p] = ptp.tile([P, ST, S], mmdt, name="pt", tag="pt")
                pts[2 * p + 1] = ptp.tile([P, ST, S], mmdt, name="pt", tag="pt")
                if p > 0:
                    dns[p - 1] = nrm.tile([P, 512], f32, name="dn", tag="dn")
                    nc.vector.memset(dns[p - 1][:], 1.0)
                if p == 0:
                    # v ones column (written once, before the V projection)
                    nc.vector.tensor_copy(
                        v_sb[:].rearrange("p i (h e) -> p i h e", e=65)[:, :, :, 64],
                        ones_f32[:, 0 : ST * H].rearrange("p (i h) -> p i h", h=H),
                    )
                for kc in range(ST):
                    if p == 0:
                        build_om(kc)
                        scores_unit(p, kc)
                        if kc < CH:
                            build_wT_chunk("wv", kc, ppr, dve_drain)
                        else:
                            proj_v_unit(2 * (kc - CH))
                            proj_v_unit(2 * (kc - CH) + 1)
                    else:
                        if kc == 0:
                            pv_start(p - 1, 0)
                        if kc == 4:
                            pv_drain(p - 1, 0, dns[p - 1])
                            pv_start(p - 1, 1)
                        scores_unit(p, kc)
                        pv_mm_unit(p - 1, kc // 4, kc % 4)
                        if p == 1:
                            # chunk-2 QK projections, spread over the slots
                            if kc == 3:
                                proj_qk_dst(2, qT, "wq", qbias_sb, 0.125, ppr, False)
                            if kc == 7:
                                proj_qk_dst(2, kT, "wk", bk_sb, 1.0, ppr, False)
                        if p == 2:
                            if kc == 3:
                                proj_qk_dst(3, qT, "wq", qbias_sb, 0.125, ppr, False)
                            if kc == 7:
                                proj_qk_dst(3, kT, "wk", bk_sb, 1.0, ppr, False)
                        if p == 3 and kc in (1, 3, 5, 7):
                            build_wT_chunk("wo", kc // 2, ppr, dve_drain)
                if p > 0:
                    pv_drain(p - 1, 1, dns[p - 1])
                    pv_norm(p - 1, dns.pop(p - 1))

            # tail: PV + norm for the last pair
            p = H // 2 - 1
            dn = nrm.tile([P, 512], f32, name="dn", tag="dn")
            nc.vector.memset(dn[:], 1.0)
            for j in range(2):
                pv_start(p, j)
                for kcp in range(CH):
                    pv_mm_unit(p, j, kcp)
                pv_drain(p, j, dn)
            pv_norm(p, dn)

            # wo^T tile home (persist pool; chunks were built in the loop)
            # -- allocated before use inside build_wT_chunk via wT dict
            # ---- output projection (psc ring; two q-tiles per tile) ----
            for qt in range(ST):
                if qt % 2 == 0:
                    psf = psc.tile([P, S], f32, name="ps", tag="ps")
                half = psf[:, (qt % 2) * 512 : (qt % 2) * 512 + 512]
                for cc in range(CH):
                    nc.tensor.matmul(
                        half,
                        oT[:, cc, qt * P : (qt + 1) * P],
                        wT["wo"][:, cc, :],
                        start=(cc == 0),
                        stop=(cc == CH - 1),
                    )
                ft = fpool.tile([P, 512], f32, name="fin", tag="fin")
                nc.vector.tensor_add(ft[:], half, bo_bc[:])
                nc.sync.dma_start(out=out_t[qt * P : (qt + 1) * P, :], in_=ft[:])

    nc.compile()
    return nc


def _get_nc():
    if "nc" not in _CACHE:
        _CACHE["nc"] = _build()
    return _CACHE["nc"]


def run(inputs, trace=False, **kw):
    from concourse.bass_utils import run_bass_kernel_spmd

    nc = _get_nc()
    f = np.float32
    in_maps = [
        {
            "x": np.ascontiguousarray(inputs["inputs"][i], dtype=f),
            "mask": np.ascontiguousarray(inputs["mask"][i], dtype=f),
            "wq": np.ascontiguousarray(inputs["wq"], dtype=f),
            "wk": np.ascontiguousarray(inputs["wk"], dtype=f),
            "wv": np.ascontiguousarray(inputs["wv"], dtype=f),
            "wo": np.ascontiguousarray(inputs["wo"], dtype=f),
            "bq": np.ascontiguousarray(inputs["bq"], dtype=f),
            "bk": np.ascontiguousarray(inputs["bk"], dtype=f),
            "bv": np.ascontiguousarray(inputs["bv"], dtype=f),
            "bo": np.ascontiguousarray(inputs["bo"], dtype=f),
        }
        for i in range(NCORES)
    ]
    res = run_bass_kernel_spmd(nc, in_maps, list(range(NCORES)), trace=trace, **kw)
    out = np.stack(
        [np.asarray(res.results[i]["out"], dtype=np.float32) for i in range(NCORES)],
        axis=0,
    )
    return out, res


def kernel(**inputs):
    out, _ = run(inputs)
    return out
